# revision 3
# baseline (speedup 1.0000x reference)
"""Fused MHA block (qkv proj + softmax(QK^T)V + out proj) for Trainium2,
SPMD across 8 NeuronCores — bf16 matmuls, latency-tuned pipeline.

Sharding: 8 cores = 2 batches x 4 head-groups (4 heads/core). Per-core partial
y's are summed on the host (b_proj added once).

vs the original baseline: same math/precision (bf16 operands, fp32 PSUM), but
- consolidated DMAs (HWDGE dispatch is ~625ns serialized per DMA): 8 input
  DMAs with xT split by T-quarters so block 0's chains start after one
  quarter; outputs batched 4 proj tiles per DMA in fp16.
- PE p-state warm-up: dummy matmuls during the input DMA wall so the 3us
  clock ramp (0.65->2.4GHz) completes before real work.
- flat software pipeline with PV lagging exp by `lag` steps; qkv chains and
  out-proj matmuls dribble through queues; softmax normalization is split
  (reciprocal+broadcast at PV end, multiply deferred a step) so the Pool
  round-trip never head-of-line-blocks the DVE queue.
- denominator trick: v_aug has a ones column FIRST, so PV emits the softmax
  denominator at PSUM partition 0 (custom-DVE reciprocal reads partition 0;
  offset 64 is a known HW bug), numerator at partitions 1-64.
- optional exp offload (pool_cnt>0): some steps' exp runs as DVE copy +
  GPSIMD pow(e, s) instead of ACT, freeing ACT when it is the bottleneck.
- out-proj drains alternate DVE/ACT at the tail flush.
"""

from collections import deque
from contextlib import ExitStack

import ml_dtypes
import numpy as np

import concourse.bass as bass  # noqa: F401
import concourse.mybir as mybir
import concourse.tile as tile
from concourse import bacc
from concourse.bass_utils import run_bass_kernel_spmd

F32 = mybir.dt.float32
F16 = mybir.dt.float16
BF16 = mybir.dt.bfloat16
FT = mybir.ActivationFunctionType
OP = mybir.AluOpType

B, D = 2, 1024
H, HD = 16, 64
NCORES = 8
HPC = 4                # heads per core
CH = HPC * HD          # 256 q/k/v channels per core
P = 128
KC = D // P            # 8 contraction chunks
SQS = float(np.sqrt(0.125))   # sqrt softmax scale, folded into wq/wk
E_CONST = float(np.e)


def build_body(tc, ctx, T, xT, wqkT, wvT, bqk, bv, wpT, yT, lag=8,
               pool_mod=4, pool_cnt=0, pool_start=16):
    nc = tc.nc
    JW = 2                 # j-chunks per pipeline step
    TI = T // 512          # 512-wide i blocks
    TJ = T // P            # 128-wide j chunks
    NSTEP = TJ // JW       # steps per block

    xT_r = xT.ap().rearrange("(kc p) t -> p kc t", p=P)
    wqk_r = wqkT.ap().rearrange("(kc p) m -> p kc m", p=P)
    wv_r = wvT.ap().rearrange("(kc p) m -> p kc m", p=P)
    wp_r = wpT.ap().rearrange("(cc p) m -> p cc m", p=P)
    yT_r = yT.ap().rearrange("(mc p) t -> p mc t", p=P)

    const = ctx.enter_context(tc.tile_pool(name="const", bufs=1))
    qkvp = ctx.enter_context(tc.tile_pool(name="qkvp", bufs=1))
    work = ctx.enter_context(tc.tile_pool(name="work", bufs=3))
    psS = ctx.enter_context(tc.tile_pool(name="psS", bufs=1, space="PSUM"))
    psO = ctx.enter_context(tc.tile_pool(name="psO", bufs=2, space="PSUM"))
    psY = ctx.enter_context(tc.tile_pool(name="psY", bufs=2, space="PSUM"))

    # ---- constants (memset only; input DMAs ordered further down) ----
    wrm = const.tile([P, 128], BF16)
    nc.gpsimd.memset(wrm, 0.0)
    e_t = const.tile([P, 512], BF16)
    nc.gpsimd.memset(e_t, E_CONST)
    bqk_sb = const.tile([P, 4], F32)
    bv_sb = const.tile([1, CH], F32)
    bvb = const.tile([P, CH], F32)
    wp_sb = const.tile([P, 2, D], BF16)

    # ---- persistent activations ----
    q_sb = qkvp.tile([P, 2, T], BF16)     # [(h%2)*64+d, h//2, i]
    k_sb = qkvp.tile([P, 2, T], BF16)
    v_sb = qkvp.tile([P, TJ, HPC * 65], BF16)  # [j, jc, h*(1+64)]
    at_sb = qkvp.tile([P, 2, T], BF16)    # attn-out^T  [ch, i]
    v4 = v_sb.rearrange("p j (h u) -> p j h u", u=65)
    nc.gpsimd.memset(v4[:, :, :, 0], 1.0)   # denominator ones col

    # ---- inputs: few big DMAs, ordered for the critical path.
    # wqk dispatches on the ACT queue in parallel with SP's boot so the
    # first big transfer starts ~1.2us earlier (before ACT's table load) ----
    xw = ctx.enter_context(tc.tile_pool(name="xw", bufs=1))
    xT_sb = xw.tile([P, KC, T], BF16)
    wqk_sb = xw.tile([P, KC, 2 * CH], BF16)
    wv_sb = xw.tile([P, KC, CH], BF16)
    nc.scalar.dma_start(out=wqk_sb, in_=wqk_r)
    # touch Exp so ACT's table loads during input DMA, not mid-pipeline
    warm = const.tile([1, 1], F32)
    nc.scalar.activation(out=warm, in_=e_t[0:1, 0:1], func=FT.Exp)
    nc.sync.dma_start(out=xT_sb[:, 0:KC // 2, 0:512],
                      in_=xT_r[:, 0:KC // 2, 0:512])
    nc.sync.dma_start(out=xT_sb[:, KC // 2:, 0:512],
                      in_=xT_r[:, KC // 2:, 0:512])
    nc.sync.dma_start(out=bqk_sb, in_=bqk.ap())
    for tq in range(1, 4):
        nc.sync.dma_start(out=xT_sb[:, :, tq * 512:(tq + 1) * 512],
                          in_=xT_r[:, :, tq * 512:(tq + 1) * 512])
    nc.sync.dma_start(out=bv_sb, in_=bv.ap())
    nc.gpsimd.partition_broadcast(out_ap=bvb, in_ap=bv_sb)
    nc.sync.dma_start(out=wv_sb, in_=wv_r)
    nc.sync.dma_start(out=wp_sb, in_=wp_r)

    # ---- PE p-state warm-up: dummy matmuls during the DMA wall ----
    for _ in range(26):
        pw = psO.tile([65, 512], F32, name="po", tag="po", bufs=2)
        nc.tensor.matmul(pw[:, 0:128], lhsT=wrm[:, 0:65], rhs=wrm,
                         start=True, stop=True)

    # ---- QKV chains (bf16, one PSUM accumulation chain each) ----
    nps = 0
    in_pipeline = False

    def qkv_ps():
        nonlocal nps
        nps += 1
        if in_pipeline or nps % 2 == 0:
            return psY.tile([P, 512], F32, name="qps", tag="py", bufs=2)
        return psS.tile([P, JW, 512], F32, name="qps", tag="st",
                        bufs=2)[:, 0, :]

    def qk_chain(ic, ch):   # ch: 0=q(h01) 1=q(h23) 2=k(h01) 3=k(h23)
        ps = qkv_ps()
        for kc in range(KC):
            nc.tensor.matmul(
                ps,
                lhsT=wqk_sb[:, kc, ch * P:(ch + 1) * P],
                rhs=xT_sb[:, kc, ic * 512:(ic + 1) * 512],
                start=(kc == 0), stop=(kc == KC - 1))
        dst = q_sb if ch < 2 else k_sb
        nc.vector.tensor_scalar(
            out=dst[:, ch % 2, ic * 512:(ic + 1) * 512], in0=ps,
            scalar1=bqk_sb[:, ch:ch + 1], scalar2=None, op0=OP.add)

    def v_chain(jc):
        ps = qkv_ps()[:, :CH]
        for kc in range(KC):
            nc.tensor.matmul(
                ps,
                lhsT=xT_sb[:, kc, jc * P:(jc + 1) * P],
                rhs=wv_sb[:, kc, :],
                start=(kc == 0), stop=(kc == KC - 1))
        nc.vector.tensor_tensor(
            out=v4[:, jc, :, 1:65],
            in0=ps.rearrange("p (h u) -> p h u", u=64),
            in1=bvb.rearrange("p (h u) -> p h u", u=64),
            op=OP.add)

    # ---- attention pipeline ----
    blocks = [(ic, h) for ic in range(TI) for h in range(HPC)]
    steps = [(bi, jj) for bi in range(len(blocks)) for jj in range(NSTEP)]
    pt_of = {}
    po_of = {}
    q_pre = deque()
    q_proj = deque()

    def emit_scores_exp(bi, jj, pool_step):
        ic, h = blocks[bi]
        hp, hh = h // 2, h % 2
        if pool_step:
            sts = [psY.tile([P, 512], F32, name="qps", tag="py", bufs=2)
                   for _ in range(JW)]
        else:
            st = psS.tile([P, JW, 512], F32, name="st", tag="st", bufs=2)
            sts = [st[:, js, :] for js in range(JW)]
        for js in range(JW):
            j = jj * JW + js
            # S^T[j, i] = sum_d k[d, j] q[d, i]  (K = 64)
            nc.tensor.matmul(
                sts[js],
                lhsT=k_sb[hh * 64:hh * 64 + 64, hp, j * P:(j + 1) * P],
                rhs=q_sb[hh * 64:hh * 64 + 64, hp,
                         ic * 512:(ic + 1) * 512],
                start=True, stop=True)
        pt = work.tile([P, JW, 512], BF16, name="pt", bufs=lag + 2)
        if pool_step:
            # DVE moves s to SBUF, GPSIMD computes pow(e, s)
            for js in range(JW):
                sc = work.tile([P, 512], BF16, name="sc", bufs=4)
                nc.vector.tensor_copy(out=sc, in_=sts[js])
                nc.gpsimd.tensor_tensor(out=pt[:, js, :], in0=e_t,
                                        in1=sc, op=OP.pow)
        else:
            nc.scalar.activation(out=pt, in_=st, func=FT.Exp)
        pt_of[(bi, jj)] = pt

    def emit_pv(bi, jj):
        ic, h = blocks[bi]
        if jj == 0:
            po_of[bi] = psO.tile([65, 512], F32, name="po", tag="po", bufs=2)
        po = po_of[bi]
        pt = pt_of.pop((bi, jj))
        for js in range(JW):
            nc.tensor.matmul(
                po,
                lhsT=v4[:, jj * JW + js, h, :],
                rhs=pt[:, js, :],
                start=(jj == 0 and js == 0),
                stop=(jj == NSTEP - 1 and js == JW - 1))

    pending_norm = deque()

    def emit_norm_head(bi):
        po = po_of.pop(bi)
        rr = work.tile([1, 512], F32, name="rr")
        nc.vector.reciprocal_approx_fast(out=rr, in_=po[0:1, :])
        rb = work.tile([64, 512], F32, name="rb", bufs=3)
        nc.gpsimd.partition_broadcast(out_ap=rb, in_ap=rr)
        pending_norm.append((bi, po, rb))

    yt_of = {}

    def defer_proj(ic):
        def mk(mc):
            def f():
                py = qkv_ps()
                for cc in range(2):
                    nc.tensor.matmul(
                        py, lhsT=wp_sb[:, cc, mc * P:(mc + 1) * P],
                        rhs=at_sb[:, cc, ic * 512:(ic + 1) * 512],
                        start=(cc == 0), stop=(cc == 1))
                mg, mi = divmod(mc, 4)
                if mi == 0:
                    yt_of[(ic, mg)] = work.tile([P, 4, 512], F16, name="yt",
                                                bufs=2)
                yt = yt_of[(ic, mg)]
                if in_pipeline or mc % 2 == 0:
                    nc.vector.tensor_copy(out=yt[:, mi, :], in_=py)
                else:
                    # tail flush: ACT is idle, split the PSUM drains
                    nc.scalar.activation(out=yt[:, mi, :], in_=py,
                                         func=FT.Identity)
                if in_pipeline:
                    if mi == 3:
                        nc.sync.dma_start(
                            out=yT_r[:, mg * 4:(mg + 1) * 4,
                                     ic * 512:(ic + 1) * 512],
                            in_=yt_of[(ic, mg)])
                elif mi % 2 == 1:
                    nc.sync.dma_start(
                        out=yT_r[:, mg * 4 + mi - 1:mg * 4 + mi + 1,
                                 ic * 512:(ic + 1) * 512],
                        in_=yt_of[(ic, mg)][:, mi - 1:mi + 1, :])
            return f
        q_proj.extend(mk(mc) for mc in range(D // P))

    def emit_norm_tail():
        bi, po, rb = pending_norm.popleft()
        ic, h = blocks[bi]
        nc.vector.tensor_tensor(
            out=at_sb[(h % 2) * 64:(h % 2) * 64 + 64, h // 2,
                      ic * 512:(ic + 1) * 512],
            in0=po[1:65, :], in1=rb, op=OP.mult)
        if h == HPC - 1:
            defer_proj(ic)

    # first block's q/k up front, split-K so the first half overlaps the
    # second xT half-DMA; the rest dribble into the pipeline.
    # k(ic) is needed by scores step 2*ic of block 0; v(jc) by PV step
    # jc//2 (idx +lag); q(ic) by block 4*ic (idx 32*ic).
    half_tmp = {}
    for half in range(2):
        for chunk in range(4):
            ps = qkv_ps()
            for kk in range(KC // 2):
                kc = half * (KC // 2) + kk
                nc.tensor.matmul(
                    ps,
                    lhsT=wqk_sb[:, kc, chunk * P:(chunk + 1) * P],
                    rhs=xT_sb[:, kc, 0:512],
                    start=(kk == 0), stop=(kk == KC // 2 - 1))
            if half == 0:
                # stage to SBUF so the PSUM slot frees before half 1
                tmp = work.tile([P, 512], F32, name="qtmp", bufs=4)
                nc.vector.tensor_copy(out=tmp, in_=ps)
                half_tmp[chunk] = tmp
            else:
                dst = q_sb if chunk < 2 else k_sb
                nc.vector.scalar_tensor_tensor(
                    out=dst[:, chunk % 2, 0:512],
                    in0=half_tmp.pop(chunk),
                    scalar=bqk_sb[:, chunk:chunk + 1],
                    in1=ps, op0=OP.add, op1=OP.add)
    for ic in range(1, TI):
        for ch in (2, 3):
            q_pre.append(lambda ic=ic, ch=ch: qk_chain(ic, ch))
    for jc in range(TJ):
        q_pre.append(lambda jc=jc: v_chain(jc))
    for ic in range(1, TI):
        for ch in (0, 1):
            q_pre.append(lambda ic=ic, ch=ch: qk_chain(ic, ch))

    in_pipeline = True
    nstep_total = len(steps)
    for idx in range(nstep_total + lag):
        while pending_norm:
            emit_norm_tail()
        if idx < nstep_total:
            emit_scores_exp(*steps[idx], pool_step=(
                pool_start <= idx < nstep_total - NSTEP
                and (idx - pool_start) % pool_mod < pool_cnt))
        # keep v-chain emission ahead of PV consumption (2 chunks/step)
        npop = 2 if q_pre else 1
        for _ in range(npop):
            if q_pre:
                q_pre.popleft()()
            elif q_proj:
                q_proj.popleft()()
        if idx >= lag:
            bi, jj = steps[idx - lag]
            emit_pv(bi, jj)
            if jj == NSTEP - 1:
                emit_norm_head(bi)
    in_pipeline = False   # flush projs may rotate through st+py slots
    while pending_norm:
        emit_norm_tail()
    while q_pre:
        q_pre.popleft()()
    while q_proj:
        q_proj.popleft()()


def build_nc(T, **kw):
    nc = bacc.Bacc("TRN2", target_bir_lowering=False, debug=False)
    xT = nc.dram_tensor("xT", [D, T], BF16, kind="ExternalInput")
    wqkT = nc.dram_tensor("wqkT", [D, 2 * CH], BF16, kind="ExternalInput")
    wvT = nc.dram_tensor("wvT", [D, CH], BF16, kind="ExternalInput")
    bqk = nc.dram_tensor("bqk", [P, 4], F32, kind="ExternalInput")
    bv = nc.dram_tensor("bv", [1, CH], F32, kind="ExternalInput")
    wpT = nc.dram_tensor("wpT", [CH, D], BF16, kind="ExternalInput")
    yT = nc.dram_tensor("yT", [D, T], F16, kind="ExternalOutput")
    with tile.TileContext(nc) as tc, ExitStack() as ctx:
        build_body(tc, ctx, T, xT, wqkT, wvT, bqk, bv, wpT, yT, **kw)
    nc.compile()
    return nc


def make_in_maps(x, w_attn, b_attn, w_proj):
    x = np.ascontiguousarray(np.asarray(x, dtype=np.float32))
    w_attn = np.asarray(w_attn, dtype=np.float32)
    b_attn = np.asarray(b_attn, dtype=np.float32)
    w_proj = np.asarray(w_proj, dtype=np.float32)
    in_maps = []
    for c in range(NCORES):
        b, g = divmod(c, HPC)
        sl = slice(g * CH, (g + 1) * CH)
        wq = w_attn[0 * D:][sl] * SQS
        wk = w_attn[1 * D:][sl] * SQS
        wv = w_attn[2 * D:][sl]
        bq = b_attn[0 * D:][sl] * SQS
        bk = b_attn[1 * D:][sl] * SQS
        bvv = b_attn[2 * D:][sl]
        wqk = np.concatenate([wq, wk], axis=0)   # [512, 1024] rows=channels
        bqk = np.stack([bq[0:128], bq[128:256], bk[0:128], bk[128:256]],
                       axis=1)
        in_maps.append({
            "xT": np.ascontiguousarray(x[b].T.astype(ml_dtypes.bfloat16)),
            "wqkT": np.ascontiguousarray(wqk.T.astype(ml_dtypes.bfloat16)),
            "wvT": np.ascontiguousarray(wv.T.astype(ml_dtypes.bfloat16)),
            "bqk": np.ascontiguousarray(bqk.astype(np.float32)),
            "bv": np.ascontiguousarray(bvv[None, :].astype(np.float32)),
            "wpT": np.ascontiguousarray(
                w_proj[:, sl].T.astype(ml_dtypes.bfloat16)),
        })
    return in_maps


_NC_CACHE = {}


def _get_nc(T):
    if T not in _NC_CACHE:
        _NC_CACHE[T] = build_nc(T)
    return _NC_CACHE[T]


def run(x, w_attn, b_attn, w_proj, b_proj, trace=False, **hw_kwargs):
    T = np.asarray(x).shape[1]
    nc = _get_nc(T)
    in_maps = make_in_maps(x, w_attn, b_attn, w_proj)
    res = run_bass_kernel_spmd(
        nc, in_maps, core_ids=list(range(NCORES)), trace=trace, **hw_kwargs
    )
    y = np.zeros((B, T, D), dtype=np.float32)
    for c in range(NCORES):
        y[c // HPC] += res.results[c]["yT"].T.astype(np.float32)
    y += np.asarray(b_proj, dtype=np.float32)
    return y, res


def kernel(x, w_attn, b_attn, w_proj, b_proj):
    y, _ = run(x, w_attn, b_attn, w_proj, b_proj)
    return y
